# revision 1
# baseline (speedup 1.0000x reference)
"""
CoordinationHistogram TRN2 kernel, v3: bin-major H masks.

Same two-level one-hot matmul as kernel.py, but the H one-hot is built
bin-major: per block of T columns, one tensor_scalar(q_tile, j, is_equal)
per bin j writes H for all T columns at once (58-cycle DVE overhead
amortized T-fold). The matmul reads H as a stride-T access pattern.
L masks stay per-column (they carry the per-edge weight z, which forces a
per-partition-scalar op); they are split DVE/GPSIMD. ACT builds the tail
bins via the exact integer one-hot Square -> Relu(1-x).
"""

import numpy as np

import concourse.tile as tile
from concourse import bacc, mybir
from concourse.bass_utils import run_bass_kernel_spmd

P = 128
NQ = 158
NATOMS = 20000
K = 16
E = 1_000_000
NCOL_FULL = 7813
TBLK = 208          # columns per block (bin-major window)
GRP = 22            # L-mask mega-tile group
DVE_BINS = 125      # H bins built on DVE; rest (33) on ACT
GPS_L = 11          # L columns per GRP built on GPSIMD (rest on DVE)

R1 = 4.4
INV2 = float(1.0 / (1.1 * 1.1))
PAD_ATOM = 20064

F32 = mybir.dt.float32
BF16 = mybir.dt.bfloat16
I32 = mybir.dt.int32
OP = mybir.AluOpType
AF = mybir.ActivationFunctionType


def _emit_cols(nc, ncol, col, blk, iota_l, mpool, coords):
    """Emit L masks + matmuls for one (deferred) block."""
    tb, rf, zf, hview = blk
    t = 0
    grp_idx = 0
    while t < tb:
        g = min(GRP, tb - t)
        gps_l = GPS_L if (grp_idx % 2 == 0) else GPS_L - 1
        if g < GRP:
            gps_l = (g * (2 * GPS_L - 1) + 2) // (2 * GRP)
        grp_idx += 1
        lmega = mpool.tile([P, GRP * P], BF16, tag="lmega")
        for i in range(g):
            lslice = lmega[:, i * P:(i + 1) * P]
            eng = nc.gpsimd if i < gps_l else nc.vector
            eng.tensor_scalar(
                lslice, iota_l[:],
                rf[:, t + i:t + i + 1], zf[:, t + i:t + i + 1],
                op0=OP.is_equal, op1=OP.mult)
        for i in range(g):
            nc.tensor.matmul(
                out=coords[:],
                lhsT=lmega[:, i * P:(i + 1) * P],
                rhs=hview[:, :, t + i],
                start=(col + i == 0), stop=(col + i == ncol - 1))
        col += g
        t += g
    return col


def build_nc(ncol=NCOL_FULL):
    e_pad = ncol * P
    nc = bacc.Bacc("TRN2", target_bir_lowering=False, debug=False)
    nv = nc.dram_tensor("nv", [e_pad * 3], F32, kind="ExternalInput")
    fa = nc.dram_tensor("fa", [e_pad], I32, kind="ExternalInput")
    out = nc.dram_tensor("out", [1, K], F32, kind="ExternalOutput")

    blocks = []
    c = 0
    while c < ncol:
        tb = min(TBLK, ncol - c)
        blocks.append((c, tb))
        c += tb

    with tile.TileContext(nc) as tc:
        with (
            tc.tile_pool(name="const", bufs=1) as cpool,
            tc.tile_pool(name="io", bufs=2) as iopool,
            tc.tile_pool(name="work", bufs=2) as wpool,
            tc.tile_pool(name="hb", bufs=2) as hpool,
            tc.tile_pool(name="mask", bufs=4) as mpool,
            tc.tile_pool(name="psum", bufs=1, space="PSUM") as ppool,
        ):
            iota_l = cpool.tile([P, P], BF16)
            nc.gpsimd.iota(iota_l[:], pattern=[[1, P]], base=0,
                           channel_multiplier=0,
                           allow_small_or_imprecise_dtypes=True)
            ones = cpool.tile([P, 1], F32)
            nc.vector.memset(ones[:], 1.0)
            bias_m1 = cpool.tile([P, 1], F32)
            nc.vector.memset(bias_m1[:], -1.0)
            bias_m4 = cpool.tile([P, 1], F32)
            nc.vector.memset(bias_m4[:], -4.0)
            bias_k = cpool.tile([P, K], F32)
            for k in range(K):
                nc.vector.memset(bias_k[:, k:k + 1], float(-k))
            # -j bias table for ACT-built bins
            bias_q = cpool.tile([P, NQ], F32)
            iq = cpool.tile([P, NQ], mybir.dt.int16)
            nc.gpsimd.iota(iq[:], pattern=[[1, NQ]], base=0,
                           channel_multiplier=0)
            nc.vector.tensor_copy(bias_q[:], iq[:])
            nc.vector.tensor_scalar(bias_q[:], bias_q[:], -1.0, None,
                                    op0=OP.mult)

            coords = ppool.tile([P, NQ], F32, space="PSUM")

            col = 0
            prev = None
            for (c0, tb) in blocks:
                ofs_e = c0 * P
                nvb = iopool.tile([P, TBLK * 3], F32, tag="nvb")
                fab = iopool.tile([P, TBLK], I32, tag="fab")
                nc.sync.dma_start(
                    nvb[:, : tb * 3],
                    nv[ofs_e * 3: (ofs_e + P * tb) * 3].rearrange(
                        "(p m) -> p m", p=P),
                )
                nc.sync.dma_start(
                    fab[:, :tb],
                    fa[ofs_e: ofs_e + P * tb].rearrange("(p m) -> p m", p=P),
                )
                v3 = nvb[:, : tb * 3].rearrange("p (m c) -> p m c", c=3)
                x, y, w = v3[:, :, 0], v3[:, :, 1], v3[:, :, 2]

                d2 = wpool.tile([P, TBLK], F32, tag="d2")
                t1 = wpool.tile([P, TBLK], F32, tag="t1")
                nc.vector.tensor_tensor(out=d2[:, :tb], in0=x, in1=x, op=OP.mult)
                nc.vector.tensor_tensor(out=t1[:, :tb], in0=y, in1=y, op=OP.mult)
                nc.vector.tensor_tensor(out=d2[:, :tb], in0=d2[:, :tb],
                                        in1=t1[:, :tb], op=OP.add)
                nc.vector.tensor_tensor(out=t1[:, :tb], in0=w, in1=w, op=OP.mult)
                nc.vector.tensor_tensor(out=d2[:, :tb], in0=d2[:, :tb],
                                        in1=t1[:, :tb], op=OP.add)
                sv = wpool.tile([P, TBLK], F32, tag="sv")
                nc.scalar.activation(sv[:, :tb], d2[:, :tb], AF.Sqrt, scale=INV2)
                y0 = wpool.tile([P, TBLK], F32, tag="y0")
                nc.scalar.activation(y0[:, :tb], sv[:, :tb], AF.Relu,
                                     bias=bias_m4[:])
                yc = wpool.tile([P, TBLK], F32, tag="yc")
                nc.vector.tensor_scalar(yc[:, :tb], y0[:, :tb], 1.0, None,
                                        op0=OP.min)
                vv = wpool.tile([P, TBLK], F32, tag="vv")
                nc.scalar.activation(vv[:, :tb], yc[:, :tb], AF.Square,
                                     bias=bias_m1[:])
                w2 = wpool.tile([P, TBLK], F32, tag="w2")
                nc.vector.tensor_scalar(w2[:, :tb], yc[:, :tb], 2.0, 1.0,
                                        op0=OP.mult, op1=OP.add)
                zf = wpool.tile([P, TBLK], F32, tag="zf")
                nc.vector.tensor_tensor(out=zf[:, :tb], in0=vv[:, :tb],
                                        in1=w2[:, :tb], op=OP.mult)
                qi = wpool.tile([P, TBLK], I32, tag="qi")
                ri = wpool.tile([P, TBLK], I32, tag="ri")
                nc.vector.tensor_scalar(qi[:, :tb], fab[:, :tb], 7, None,
                                        op0=OP.logical_shift_right)
                nc.vector.tensor_scalar(ri[:, :tb], fab[:, :tb], 127, None,
                                        op0=OP.bitwise_and)
                qf = wpool.tile([P, TBLK], F32, tag="qf")
                rf = wpool.tile([P, TBLK], F32, tag="rf")
                nc.vector.tensor_copy(qf[:, :tb], qi[:, :tb])
                nc.vector.tensor_copy(rf[:, :tb], ri[:, :tb])
                qb = wpool.tile([P, TBLK], BF16, tag="qb")
                nc.vector.tensor_copy(qb[:, :tb], qf[:, :tb])

                # ---- software pipeline: emit L masks + matmuls for the
                # PREVIOUS block first (PE gets lhsT tiles early in this
                # engine section), then this block's bins — which therefore
                # complete a full block ahead of their matmuls and never
                # gate the PE.
                if prev is not None:
                    col = _emit_cols(nc, ncol, col, prev, iota_l, mpool, coords)

                # ---- bin-major H: HB[p, j*TBLK + t] = (q[p,t] == j) ----
                hb = hpool.tile([P, NQ * TBLK], BF16, tag="hb")
                for j in range(NQ):
                    hslice = hb[:, j * TBLK: j * TBLK + tb]
                    if j < DVE_BINS:
                        nc.vector.tensor_scalar(hslice, qb[:, :tb], float(j),
                                                None, op0=OP.is_equal)
                    else:
                        hsq = mpool.tile([P, TBLK], BF16, tag="hsq")
                        nc.scalar.activation(hsq[:, :tb], qb[:, :tb],
                                             AF.Square,
                                             bias=bias_q[:, j:j + 1])
                        nc.scalar.activation(hslice, hsq[:, :tb], AF.Relu,
                                             bias=ones[:], scale=-1.0)
                hview = hb[:].rearrange("p (j t) -> p j t", t=TBLK)
                prev = (tb, rf, zf, hview)
            col = _emit_cols(nc, ncol, col, prev, iota_l, mpool, coords)

            # ---- KDE ----
            acc1 = cpool.tile([P, K], F32)
            acc2 = cpool.tile([32, K], F32)
            sq = wpool.tile([P, NQ], F32, tag="sq")
            ek = wpool.tile([P, NQ], F32, tag="ek")
            for k in range(K):
                nc.scalar.activation(sq[:], coords[:], AF.Square,
                                     bias=bias_k[:, k:k + 1])
                nc.scalar.activation(ek[:, :156], sq[:, :156], AF.Exp,
                                     scale=-2.0, accum_out=acc1[:, k:k + 1])
                nc.scalar.activation(ek[0:32, 156:157], sq[0:32, 156:157],
                                     AF.Exp, scale=-2.0,
                                     accum_out=acc2[:, k:k + 1])
            hist_ps = ppool.tile([1, K], F32, space="PSUM")
            nc.tensor.matmul(out=hist_ps[:], lhsT=ones[:], rhs=acc1[:],
                             start=True, stop=False)
            nc.tensor.matmul(out=hist_ps[:], lhsT=ones[0:32, :], rhs=acc2[:],
                             start=False, stop=True)
            res = cpool.tile([1, K], F32)
            nc.vector.tensor_copy(res[:], hist_ps[:])
            nc.sync.dma_start(out[:], res[:])
    nc.compile()
    return nc


def _shard_inputs(neighbor_vectors, first_atom, ncol=NCOL_FULL):
    e_pad = ncol * P
    s = neighbor_vectors.shape[0]
    in_maps = []
    for i in range(s):
        nvs = np.asarray(neighbor_vectors[i], dtype=np.float32).reshape(-1, 3)
        fas = np.asarray(first_atom[i], dtype=np.int32).reshape(-1)
        n = min(e_pad, nvs.shape[0])
        nv_pad = np.empty((e_pad, 3), dtype=np.float32)
        nv_pad[:n] = nvs[:n]
        nv_pad[n:] = np.array([10.0, 0.0, 0.0], dtype=np.float32)
        fa_pad = np.full((e_pad,), PAD_ATOM, dtype=np.int32)
        fa_pad[:n] = fas[:n]
        in_maps.append({"nv": nv_pad.reshape(-1), "fa": fa_pad})
    return in_maps


def run(neighbor_vectors, first_atom, ncol=NCOL_FULL, trace=False):
    nc = build_nc(ncol)
    in_maps = _shard_inputs(neighbor_vectors, first_atom, ncol)
    br = run_bass_kernel_spmd(nc, in_maps, core_ids=list(range(len(in_maps))),
                              trace=trace)
    out = np.stack([br.results[i]["out"][0] for i in range(len(in_maps))])
    return out.astype(np.float32), br


def kernel(neighbor_vectors, first_atom):
    out, _ = run(neighbor_vectors, first_atom)
    return out

